# revision 7
# baseline (speedup 1.0000x reference)
"""Trainium2 Bass kernel: Based linear attention (poly feature map, causal, normalized).

Full inputs q,k,v: [1, 16, 4096, 16] fp32. Output: [1, 16, 4096, 16] fp32.
Sharding: 16 heads over 8 cores (2 heads/core); each head is independent.

Algorithm (per head): chunked quadratic-state linear attention.
  scores s = scale * q.k ; poly P = 1 + s + 0.5 s^2 = 0.5(1+s)^2 + 0.5, causal.
  With psi_k = [1, k] (17-dim), chi_a = [1, scale*q]/sqrt(2):
    0.5(1+s)^2 = (chi_a . psi_k)^2  -> quadratic state T[(r,p), d'] = sum_j psi_k_r psi_k_p v'_d
    (v' = [v, 1] carries the normalizer z in channel 16)
  Per 128-position chunk: diagonal block computed directly (matmul + Square + mask);
  the "+0.5" causal term via a constant 0.5*triangular matmul; cross-chunk part via the
  289-dim state: G = chi_a^T T, H = G * bcast(chi_a), num2 = group-reduce(H).
  The extra +0.5*prefix(v') term folds into the state by doubling the (0,0) feature row,
  baked into the update matmul through a [2|k|0..] variant of the k feature tensor.
  State is kept replicated at partition bases {0,32,64,96} (overlap access pattern on the
  update matmul's lhsT) so per-chunk transposed feature tiles can matmul it directly.
"""
import numpy as np
from contextlib import ExitStack

import concourse.bass as bass
import concourse.bacc as bacc
import concourse.tile as tile
import concourse.mybir as mybir
from bass_rust import add_dep_helper
from concourse.masks import make_identity, make_upper_triangular
from concourse.bass_utils import run_bass_kernel_spmd

B, H, S, D = 1, 16, 4096, 16
NCORES = 8
HPC = H // NCORES  # heads per core
C = 128            # chunk (positions)
NCH = S // C       # 32 chunks
D1 = D + 1         # 17
F = D1 * D1        # 289
dt = mybir.dt.float32
bt = mybir.dt.bfloat16
SCALE = 1.0 / np.sqrt(D)
SC = SCALE / np.sqrt(2.0)
RT2I = 1.0 / np.sqrt(2.0)


def _fd(ap, offset_ap, dims):
    """AP on the same tensor as `ap`, partition dim kept, free dims replaced."""
    return bass.AP(tensor=ap.tensor, offset=offset_ap.offset, ap=[ap.ap[0]] + dims)


def _build_head(nc, tc, ctx, pools, h, q_d, k_d, v_d, o_d):
    const, bulk, sb, st, ps128, psG, psN, psT = pools
    ident, mask, trih = const

    # ---- raw loads: [S, D] -> [128, NCH, D] (position p on partitions) ----
    qraw = bulk.tile([128, NCH, D], dt, tag="qraw")
    kraw = bulk.tile([128, NCH, D], dt, tag="kraw")
    vraw = bulk.tile([128, NCH, D], dt, tag="vraw")
    nc.sync.dma_start(qraw[:], q_d[h].rearrange("(c p) d -> p c d", p=128))
    nc.sync.dma_start(kraw[:], k_d[h].rearrange("(c p) d -> p c d", p=128))
    nc.sync.dma_start(vraw[:], v_d[h].rearrange("(c p) d -> p c d", p=128))

    # ---- bulk feature tensors ----
    # kb:  [1 | k | 0*15] per chunk (stride 32)
    # kb2: [2 | k | 0*15] per chunk (doubled homogeneous row for the state update)
    # ab:  [1/sqrt2 | sc*q | 0*15]
    # vb:  [v | 1] (stride 17)
    kb = bulk.tile([128, NCH, 32], bt, tag="kb")
    ab = bulk.tile([128, NCH, 32], bt, tag="ab")
    vb = bulk.tile([128, NCH, D1], bt, tag="vb")
    kb4 = bulk.tile([128, NCH, 128], bt, tag="kb4")  # [1|k|0*15] x4 replicated
    nc.gpsimd.memset(kb[:], 0.0)
    nc.gpsimd.memset(ab[:], 0.0)
    nc.gpsimd.memset(kb4[:], 0.0)
    nc.vector.memset(kb[:, :, 0:1], 1.0)
    nc.vector.memset(ab[:, :, 0:1], RT2I)
    nc.vector.memset(vb[:, :, D : D + 1], 1.0)
    nc.scalar.copy(kb[:, :, 1 : D + 1], kraw[:])
    nc.scalar.mul(ab[:, :, 1 : D + 1], qraw[:], SC)
    nc.vector.tensor_copy(vb[:, :, 0:D], vraw[:])
    kb4_r = kb4[:].rearrange("p c (r e) -> p c r e", r=4)
    nc.vector.memset(kb4_r[:, :, :, 0:1], 1.0)
    kraw_bc = bass.AP(tensor=kraw[:].tensor, offset=kraw[:].offset,
                      ap=[kraw[:].ap[0], [D, NCH], [0, 4], [1, D]])
    nc.vector.tensor_copy(kb4_r[:, :, :, 1 : D + 1], kraw_bc)

    # ---- transposed feature tiles, 4 chunks per [128,128] at bases {0,32,64,96} ----
    ktp = bulk.tile([128, NCH // 4, 128], bt, tag="ktp")
    atp = bulk.tile([128, NCH // 4, 128], bt, tag="atp")
    for g in range(NCH // 4):
        kt_ps = ps128.tile([128, 128], dt, tag="st")
        at_ps = ps128.tile([128, 128], dt, tag="st")
        kb_slab = _fd(kb[:], kb[:, 4 * g, 0:1], [[1, 128]])
        ab_slab = _fd(ab[:], ab[:, 4 * g, 0:1], [[1, 128]])
        nc.tensor.matmul(kt_ps[:], kb_slab, ident[:], start=True, stop=True)
        nc.tensor.matmul(at_ps[:], ab_slab, ident[:], start=True, stop=True)
        # alternate engines for the PSUM->SBUF evacuation
        if g % 2 == 0:
            nc.scalar.copy(ktp[:, g, :], kt_ps[:])
            nc.vector.tensor_copy(atp[:, g, :], at_ps[:])
        else:
            nc.vector.tensor_copy(ktp[:, g, :], kt_ps[:])
            nc.scalar.copy(atp[:, g, :], at_ps[:])

    o_sb = bulk.tile([128, NCH, D], dt, tag="osb")
    t4 = psT.tile([128, F], dt, tag="t4")
    nc.vector.memset(t4[:], 0.0)
    t4sb_prev = None
    prev_copy = None

    for c in range(NCH):
        g, b = divmod(c, 4)
        p0 = 32 * b

        # intra scores (transposed): S_T[j, q] = psi_k_j . chi_a_q
        st_ps = ps128.tile([128, 128], dt, tag="st")
        nc.tensor.matmul(st_ps[:], ktp[p0 : p0 + D1, g, :], atp[p0 : p0 + D1, g, :],
                         start=True, stop=True, tile_position=(p0, 0))
        sq = sb.tile([128, 128], bt, tag="sq")
        nc.scalar.activation(sq[:], st_ps[:], mybir.ActivationFunctionType.Square)
        pt = sb.tile([128, 128], bt, tag="pt")
        nc.vector.scalar_tensor_tensor(
            pt[:], sq[:], 0.5, mask[:], mybir.AluOpType.add, mybir.AluOpType.mult
        )

        # intra numerator (+z): num[q, d'] = sum_{j<=q} (sq+0.5)[j,q] v'[j,d']
        num_ps = psN.tile([128, D1], dt, tag="num")
        nc.tensor.matmul(num_ps[:], pt[:], vb[:, c, :], start=True, stop=True)

        tot = sb.tile([128, D1], dt, tag="tot")
        if c > 0:
            g_ps = psG.tile([128, F], dt, tag="g")
            nc.tensor.matmul(g_ps[:], atp[p0 : p0 + D1, g, :],
                             t4sb_prev[p0 : p0 + D1, :],
                             start=True, stop=True, tile_position=(p0, 0))
            h_t = sb.tile([128, F], bt, tag="h")
            ab_bc = _fd(ab[:], ab[:, c, 0:1], [[1, D1], [0, D1]])
            nc.vector.scalar_tensor_tensor(
                h_t[:], g_ps[:], 1.0, ab_bc, mybir.AluOpType.mult, mybir.AluOpType.mult
            )
            num2 = sb.tile([128, D1], dt, tag="num2")
            h_r = _fd(h_t[:], h_t[:], [[1, D1], [D1, D1]])
            nc.vector.tensor_reduce(num2[:], h_r, axis=mybir.AxisListType.X,
                                    op=mybir.AluOpType.add)
            nc.vector.scalar_tensor_tensor(
                tot[:], num_ps[:], 1.0, num2[:], mybir.AluOpType.mult,
                mybir.AluOpType.add
            )
        else:
            nc.scalar.copy(tot[:], num_ps[:])

        # normalize: out = num / z
        rec = sb.tile([128, 1], dt, tag="rec")
        nc.vector.reciprocal(rec[:], tot[:, D : D + 1])
        nc.scalar.activation(o_sb[:, c, :], tot[:, 0:D],
                             mybir.ActivationFunctionType.Copy, scale=rec[:])

        # state update (4-base replicated): T4 += [psi~k | psi_k (x) W] per position
        w = sb.tile([128, D * D1], bt, tag="w")
        kb_bc = _fd(kb[:], kb[:, c, 1 : 1 + D], [[1, D], [0, D1]])
        vb_bc = _fd(vb[:], vb[:, c, 0:1], [[0, D], [1, D1]])
        nc.gpsimd.tensor_mul(w[:], kb_bc, vb_bc)
        mm1 = nc.tensor.matmul(t4[:, 0:D1], kb4[:, c, :], vb[:, c, :],
                               start=False, stop=False, skip_group_check=True)
        mm2 = nc.tensor.matmul(t4[:, D1:F], kb4[:, c, :], w[:],
                               start=False, stop=False, skip_group_check=True)
        dbl = []
        for bb in range(4):
            d_mm = nc.tensor.matmul(t4[32 * bb : 32 * bb + 1, 0:D1],
                                    kb4[:, c, 0:1], vb[:, c, :],
                                    start=False, stop=False,
                                    tile_position=(0, 32 * bb),
                                    skip_group_check=True)
            dbl.append(d_mm)
        if prev_copy is not None:
            # accumulating matmuls are treated as commutative by the scheduler;
            # fence them against the state snapshot reads explicitly
            for m in [mm1, mm2] + dbl:
                add_dep_helper(m.ins, prev_copy.ins, reason="t4 update after snapshot")

        if c < NCH - 1:
            t4sb = st.tile([128, F], bt, tag="t4sb")
            cp = nc.scalar.copy(t4sb[:], t4[:])
            for m in [mm1, mm2] + dbl:
                add_dep_helper(cp.ins, m.ins, reason="snapshot after t4 update")
            t4sb_prev = t4sb
            prev_copy = cp

    nc.sync.dma_start(o_d[h].rearrange("(c p) d -> p c d", p=128), o_sb[:])


def build_program():
    nc = bacc.Bacc("TRN2", target_bir_lowering=False, debug=False)
    q_d = nc.dram_tensor("q", [HPC, S, D], dt, kind="ExternalInput")
    k_d = nc.dram_tensor("k", [HPC, S, D], dt, kind="ExternalInput")
    v_d = nc.dram_tensor("v", [HPC, S, D], dt, kind="ExternalInput")
    o_d = nc.dram_tensor("out", [HPC, S, D], dt, kind="ExternalOutput")

    with tile.TileContext(nc) as tc, ExitStack() as ctx:
        constp = ctx.enter_context(tc.tile_pool(name="const", bufs=1))
        bulk = ctx.enter_context(tc.tile_pool(name="bulk", bufs=2))
        sb = ctx.enter_context(tc.tile_pool(name="sb", bufs=3))
        st = ctx.enter_context(tc.tile_pool(name="st", bufs=2))
        ps128 = ctx.enter_context(tc.tile_pool(name="ps128", bufs=2, space="PSUM"))
        psG = ctx.enter_context(tc.tile_pool(name="psG", bufs=2, space="PSUM"))
        psN = ctx.enter_context(tc.tile_pool(name="psN", bufs=2, space="PSUM"))
        psT = ctx.enter_context(tc.tile_pool(name="psT", bufs=1, space="PSUM"))

        ident = constp.tile([128, 128], bt)
        make_identity(nc, ident)
        mask = constp.tile([128, 128], bt)
        make_upper_triangular(nc, mask, val=1.0, diag=True)
        pools = ((ident, mask, None), bulk, sb, st, ps128, psG, psN, psT)
        for h in range(HPC):
            _build_head(nc, None, ctx, pools, h, q_d, k_d, v_d, o_d)

    nc.compile()
    return nc


_NC = None


def kernel(q: np.ndarray, k: np.ndarray, v: np.ndarray) -> np.ndarray:
    global _NC
    if _NC is None:
        _NC = build_program()
    q = np.ascontiguousarray(np.asarray(q, dtype=np.float32).reshape(H, S, D))
    k = np.ascontiguousarray(np.asarray(k, dtype=np.float32).reshape(H, S, D))
    v = np.ascontiguousarray(np.asarray(v, dtype=np.float32).reshape(H, S, D))
    in_maps = []
    for i in range(NCORES):
        sl = slice(i * HPC, (i + 1) * HPC)
        in_maps.append({
            "q": np.ascontiguousarray(q[sl]),
            "k": np.ascontiguousarray(k[sl]),
            "v": np.ascontiguousarray(v[sl]),
        })
    res = run_bass_kernel_spmd(_NC, in_maps, core_ids=list(range(NCORES)))
    outs = [res.results[i]["out"] for i in range(NCORES)]
    return np.concatenate(outs, axis=0).reshape(B, H, S, D)


# revision 14
# speedup vs baseline: 1.0503x; 1.0503x over previous
"""Trainium2 Bass kernel: Based linear attention (poly feature map, causal, normalized).

Full inputs q,k,v: [1, 16, 4096, 16] fp32. Output: [1, 16, 4096, 16] fp32.
Sharding: 16 heads over 8 cores (2 heads/core); each head is independent.

Algorithm (per head): chunked quadratic-state linear attention.
  scores s = scale * q.k ; poly P = 1 + s + 0.5 s^2 = (chi_a . psi_k)^2 + 0.5
  with psi_k = [1, k] (17-dim), chi_a = [1, scale*q]/sqrt(2).
  Quadratic state T[(r,p), d'] = sum_j psi_k_r psi_k_p v'_d with v' = [v, 1]
  (channel 16 carries the normalizer z). Per 128-position chunk: the diagonal
  block is computed directly (matmul + ACT Square + fused (sq+0.5)*mask);
  the cross-chunk part contracts query features against the state
  (G = chi_a^T T, H = G * bcast(chi_a), num2 = 17-group reduce of H).
  The +0.5*prefix(v') term is folded in by doubling the state's (0,0) feature
  row (4 tiny accumulating matmuls). The state is replicated at partition
  bases {0,32,64,96} via a materialized 4x-replicated k-feature tensor so the
  per-chunk transposed feature tiles (4 chunks packed per PE transpose) can
  matmul it directly. The two heads of a core are processed interleaved with
  their elementwise work fused into paired wide ops (halves fixed overheads).
"""
import numpy as np
from contextlib import ExitStack

import concourse.bass as bass
import concourse.bacc as bacc
import concourse.tile as tile
import concourse.mybir as mybir
from bass_rust import add_dep_helper
from concourse.masks import make_identity, make_upper_triangular
from concourse.bass_utils import run_bass_kernel_spmd

B, H, S, D = 1, 16, 4096, 16
NCORES = 8
HPC = H // NCORES  # heads per core (2)
C = 128            # chunk (positions)
NCH = S // C       # 32 chunks
D1 = D + 1         # 17
F = D1 * D1        # 289
dt = mybir.dt.float32
bt = mybir.dt.bfloat16
SCALE = 1.0 / np.sqrt(D)
SC = SCALE / np.sqrt(2.0)
RT2I = 1.0 / np.sqrt(2.0)


def _fd(ap, offset_ap, dims):
    """AP on the same tensor as `ap`, partition dim kept, free dims replaced."""
    return bass.AP(tensor=ap.tensor, offset=offset_ap.offset, ap=[ap.ap[0]] + dims)


def _build_core(nc, pools, q_d, k_d, v_d, o_d):
    (ident, mask), bulk, sb, st, ps128, psG, psN, psT = pools

    # ---- raw loads (both heads): [h, S, D] -> [128, h, NCH, D] ----
    qraw = bulk.tile([128, HPC, NCH, D], dt, tag="qraw")
    kraw = bulk.tile([128, HPC, NCH, D], dt, tag="kraw")
    vraw = bulk.tile([128, HPC, NCH, D], dt, tag="vraw")
    for h in range(HPC):
        nc.sync.dma_start(qraw[:, h], q_d[h].rearrange("(c p) d -> p c d", p=128))
        nc.sync.dma_start(kraw[:, h], k_d[h].rearrange("(c p) d -> p c d", p=128))
        nc.sync.dma_start(vraw[:, h], v_d[h].rearrange("(c p) d -> p c d", p=128))

    # ---- paired bulk feature tensors ----
    # kb: [1|k|0*15] per chunk; ab: [1/sqrt2|sc*q|0*15]; vb: [v|1]
    # kb4: [1|k|0*15] x4 replicated (state-update lhsT)
    kb = bulk.tile([128, HPC, NCH, 32], bt, tag="kb")
    ab = bulk.tile([128, HPC, NCH, 32], bt, tag="ab")
    vb = bulk.tile([128, HPC, NCH, D1], bt, tag="vb")
    kb4 = bulk.tile([128, HPC, NCH, 128], bt, tag="kb4")
    nc.gpsimd.memset(kb[:], 0.0)
    nc.gpsimd.memset(ab[:], 0.0)
    nc.gpsimd.memset(kb4[:], 0.0)
    nc.vector.memset(kb[:, :, :, 0:1], 1.0)
    nc.vector.memset(ab[:, :, :, 0:1], RT2I)
    nc.vector.memset(vb[:, :, :, D : D + 1], 1.0)
    nc.scalar.copy(kb[:, :, :, 1 : D + 1], kraw[:])
    nc.scalar.mul(ab[:, :, :, 1 : D + 1], qraw[:], SC)
    nc.vector.tensor_copy(vb[:, :, :, 0:D], vraw[:])
    kb4_r = kb4[:].rearrange("p h c (r e) -> p h c r e", r=4)
    nc.vector.memset(kb4_r[:, :, :, :, 0:1], 1.0)
    for h in range(HPC):
        kraw_bc = bass.AP(tensor=kraw[:].tensor, offset=kraw[:, h].offset,
                          ap=[kraw[:].ap[0], [D, NCH], [0, 4], [1, D]])
        nc.vector.tensor_copy(kb4_r[:, h, :, :, 1 : D + 1], kraw_bc)

    # ---- transposed feature tiles: 4 chunks per [128,128] at bases {0,32,64,96} ----
    ktp = bulk.tile([128, HPC, NCH // 4, 128], bt, tag="ktp")
    atp = bulk.tile([128, HPC, NCH // 4, 128], bt, tag="atp")
    for h in range(HPC):
        for g in range(NCH // 4):
            kt_ps = ps128.tile([128, 2, 128], dt, tag="st")
            kb_slab = _fd(kb[:], kb[:, h, 4 * g, 0:1], [[1, 128]])
            ab_slab = _fd(ab[:], ab[:, h, 4 * g, 0:1], [[1, 128]])
            nc.tensor.matmul(kt_ps[:, 0, :], kb_slab, ident[:], start=True, stop=True)
            nc.tensor.matmul(kt_ps[:, 1, :], ab_slab, ident[:], start=True, stop=True)
            if g % 2 == 0:
                nc.scalar.copy(ktp[:, h, g, :], kt_ps[:, 0, :])
                nc.vector.tensor_copy(atp[:, h, g, :], kt_ps[:, 1, :])
            else:
                nc.vector.tensor_copy(ktp[:, h, g, :], kt_ps[:, 0, :])
                nc.scalar.copy(atp[:, h, g, :], kt_ps[:, 1, :])

    o_sb = bulk.tile([128, HPC, NCH, D], dt, tag="osb")
    # paired state PSUM: head h at column offset 512*h (bank-aligned)
    t4p = psT.tile([128, HPC, 512], dt, tag="t4")
    nc.vector.memset(t4p[:], 0.0)
    t4sb_prev = None
    prev_copy = None

    for c in range(NCH):
        g, b = divmod(c, 4)
        p0 = 32 * b

        # paired intra scores: S_T[j, (h,q)]
        stp = ps128.tile([128, HPC, 128], dt, tag="st")
        for h in range(HPC):
            nc.tensor.matmul(stp[:, h, :], ktp[p0 : p0 + D1, h, g, :],
                             atp[p0 : p0 + D1, h, g, :],
                             start=True, stop=True, tile_position=(p0, 0))
        sq = sb.tile([128, HPC, 128], bt, tag="sq")
        nc.scalar.activation(sq[:], stp[:], mybir.ActivationFunctionType.Square)
        pt = sb.tile([128, HPC, 128], bt, tag="pt")
        mask_bc = _fd(mask[:], mask[:], [[0, HPC], [1, 128]])
        nc.vector.scalar_tensor_tensor(
            pt[:], sq[:], 0.5, mask_bc, mybir.AluOpType.add, mybir.AluOpType.mult
        )

        num_ps = psN.tile([128, HPC, D1], dt, tag="num")
        for h in range(HPC):
            nc.tensor.matmul(num_ps[:, h, :], pt[:, h, :], vb[:, h, c, :],
                             start=True, stop=True)

        tot = sb.tile([128, HPC, D1], dt, tag="tot")
        if c > 0:
            gp = psG.tile([128, HPC, 512], dt, tag="g")
            for h in range(HPC):
                nc.tensor.matmul(gp[:, h, 0:F], atp[p0 : p0 + D1, h, g, :],
                                 t4sb_prev[p0 : p0 + D1, h, :],
                                 start=True, stop=True, tile_position=(p0, 0))
            h_t = sb.tile([128, HPC, F], bt, tag="h")
            num2 = sb.tile([128, HPC, D1], dt, tag="num2")
            for h in range(HPC):
                ab_bc = _fd(ab[:], ab[:, h, c, 0:1], [[1, D1], [0, D1]])
                nc.vector.scalar_tensor_tensor(
                    h_t[:, h, :], gp[:, h, 0:F], 1.0, ab_bc,
                    mybir.AluOpType.mult, mybir.AluOpType.mult
                )
                h_r = _fd(h_t[:], h_t[:, h, 0:1], [[1, D1], [D1, D1]])
                nc.vector.tensor_reduce(num2[:, h, :], h_r,
                                        axis=mybir.AxisListType.X,
                                        op=mybir.AluOpType.add)
            nc.vector.scalar_tensor_tensor(
                tot[:], num_ps[:], 1.0, num2[:], mybir.AluOpType.mult,
                mybir.AluOpType.add
            )
        else:
            nc.scalar.copy(tot[:], num_ps[:])

        # normalize: out = num / z (z = channel 16)
        rec = sb.tile([128, HPC, 1], dt, tag="rec")
        nc.vector.reciprocal(rec[:], tot[:, :, D : D + 1])
        for h in range(HPC):
            nc.scalar.activation(o_sb[:, h, c, :], tot[:, h, 0:D],
                                 mybir.ActivationFunctionType.Copy,
                                 scale=rec[:, h, :])

        # state update: T += [psi_k | psi_k (x) W] per position, 4-base replicated
        w = sb.tile([128, HPC, D * D1], bt, tag="w")
        for h in range(HPC):
            kb_bc = _fd(kb[:], kb[:, h, c, 1 : 1 + D], [[1, D], [0, D1]])
            vb_bc = _fd(vb[:], vb[:, h, c, 0:1], [[0, D], [1, D1]])
            nc.gpsimd.tensor_mul(w[:, h, :], kb_bc, vb_bc)
        mms = []
        for h in range(HPC):
            mms.append(nc.tensor.matmul(t4p[:, h, 0:D1], kb4[:, h, c, :],
                                        vb[:, h, c, :], start=False, stop=False,
                                        skip_group_check=True))
            mms.append(nc.tensor.matmul(t4p[:, h, D1:F], kb4[:, h, c, :],
                                        w[:, h, :], start=False, stop=False,
                                        skip_group_check=True))
            for bb in range(4):
                mms.append(nc.tensor.matmul(
                    t4p[32 * bb : 32 * bb + 1, h, 0:D1],
                    kb4[:, h, c, 0:1], vb[:, h, c, :],
                    start=False, stop=False, tile_position=(0, 32 * bb),
                    skip_group_check=True))
        if prev_copy is not None:
            for m in mms:
                add_dep_helper(m.ins, prev_copy.ins, reason="t4 update after snapshot")

        if c < NCH - 1:
            t4sb = st.tile([128, HPC, F], bt, tag="t4sb")
            cp = nc.scalar.copy(t4sb[:], t4p[:, :, 0:F])
            for m in mms:
                add_dep_helper(cp.ins, m.ins, reason="snapshot after t4 update")
            t4sb_prev = t4sb
            prev_copy = cp

    for h in range(HPC):
        nc.sync.dma_start(o_d[h].rearrange("(c p) d -> p c d", p=128), o_sb[:, h])


def build_program():
    nc = bacc.Bacc("TRN2", target_bir_lowering=False, debug=False)
    q_d = nc.dram_tensor("q", [HPC, S, D], dt, kind="ExternalInput")
    k_d = nc.dram_tensor("k", [HPC, S, D], dt, kind="ExternalInput")
    v_d = nc.dram_tensor("v", [HPC, S, D], dt, kind="ExternalInput")
    o_d = nc.dram_tensor("out", [HPC, S, D], dt, kind="ExternalOutput")

    with tile.TileContext(nc) as tc, ExitStack() as ctx:
        constp = ctx.enter_context(tc.tile_pool(name="const", bufs=1))
        bulk = ctx.enter_context(tc.tile_pool(name="bulk", bufs=1))
        sb = ctx.enter_context(tc.tile_pool(name="sb", bufs=3))
        st = ctx.enter_context(tc.tile_pool(name="st", bufs=2))
        ps128 = ctx.enter_context(tc.tile_pool(name="ps128", bufs=2, space="PSUM"))
        psG = ctx.enter_context(tc.tile_pool(name="psG", bufs=1, space="PSUM"))
        psN = ctx.enter_context(tc.tile_pool(name="psN", bufs=2, space="PSUM"))
        psT = ctx.enter_context(tc.tile_pool(name="psT", bufs=1, space="PSUM"))

        ident = constp.tile([128, 128], bt)
        make_identity(nc, ident)
        mask = constp.tile([128, 128], bt)
        make_upper_triangular(nc, mask, val=1.0, diag=True)

        pools = ((ident, mask), bulk, sb, st, ps128, psG, psN, psT)
        _build_core(nc, pools, q_d, k_d, v_d, o_d)

    nc.compile()
    return nc


_NC = None


def kernel(q: np.ndarray, k: np.ndarray, v: np.ndarray) -> np.ndarray:
    global _NC
    if _NC is None:
        _NC = build_program()
    q = np.ascontiguousarray(np.asarray(q, dtype=np.float32).reshape(H, S, D))
    k = np.ascontiguousarray(np.asarray(k, dtype=np.float32).reshape(H, S, D))
    v = np.ascontiguousarray(np.asarray(v, dtype=np.float32).reshape(H, S, D))
    in_maps = []
    for i in range(NCORES):
        sl = slice(i * HPC, (i + 1) * HPC)
        in_maps.append({
            "q": np.ascontiguousarray(q[sl]),
            "k": np.ascontiguousarray(k[sl]),
            "v": np.ascontiguousarray(v[sl]),
        })
    res = run_bass_kernel_spmd(_NC, in_maps, core_ids=list(range(NCORES)))
    outs = [res.results[i]["out"] for i in range(NCORES)]
    return np.concatenate(outs, axis=0).reshape(B, H, S, D)
